# revision 1
# baseline (speedup 1.0000x reference)
"""Sliding-window GQA attention block (RoPE + QKV proj + SWA + out proj) on 8
Trainium2 NeuronCores.

Sharding: batch (2) x sequence chunks (4 x 512) -> 8 cores, SPMD. Each core
computes a 512-query slice of the output using a 192-position K/V halo, so no
cross-core reduction is needed; outputs concatenate exactly.

Per-core dataflow (activations transposed, head-dim on partitions; all DMA'd
operands fp16 to halve HBM traffic -- fp16 moving operands run at the full
1 cycle/row PE rate at any width, so the projection matmuls are unaffected):
  qT/kT = W^T-tiled matmuls vs fp16 xT, RoPE applied via sign-folded fp16 sin
  tables (rotate-half via per-operand partition bases, adds on GpSimd);
  attention runs group-major on 64-query blocks whose 257-wide sliding window
  tiles into exactly two 128-row kv chunks computed into one [128,512] PSUM
  bank, so a single Exp (ACT, fp16 out) covers both chunks; the band+validity
  mask is a binary fp16 multiply on the otherwise-idle GpSimd (SBUF-only, as
  GpSimd cannot touch PSUM); softmax sums come from a ones-matrix matmul
  replicating column sums across partitions, and normalization runs as
  reciprocal+multiply on DVE, paired across two iterations sharing psum banks
  to halve its cost (a DVE op may read at most one PSUM operand, ruling out
  a direct divide). Scores/exp/mask are emitted LEAD iterations ahead of the
  consuming sums/AV matmuls, and the q-head/K projections feeding attention
  group g+2 ride between group g's iterations, so the PE queue never drains
  into a phase boundary. AV packs the 4 query heads of a KV group into one
  256-wide moving operand (a 64-shifted V copy serves the odd blocks); wo
  consumes the fp16 [d, pos] attention layout directly, with its fp16 weight
  tiles prefetched during attention; the fp16 output halves the store DMAs.
  Softmax skips max-subtraction (scores are bounded small).
"""
import numpy as np

import concourse.tile as tile
from concourse import bacc, bass_isa, mybir
from concourse.bass_utils import run_bass_kernel_spmd

F32 = mybir.dt.float32
F32R = mybir.dt.float32r
F16 = mybir.dt.float16

B, L, DIM = 2, 2048, 2048
NH, NKV, HD, W = 16, 4, 128, 192
LQ, HALO = 512, 192
LK = LQ + HALO              # 704
KT = DIM // 128             # 16 contraction tiles
NQB = LQ // 128             # 4 query blocks
NVT = (LK + 127) // 128     # 6 value pos-tiles
SCALE = HD ** -0.5
GRP = NH // NKV             # 4 query heads per kv head

_CACHE = {}

# tuning knobs (fixed to the best measured config; module-level so the
# experiment harness can sweep them)
TAIL_SPLIT = False   # last wo block in column slices (measured slower)
TAIL_NARROW = False  # last wo block as 384+128 so the final copy is short
PST_BUFS = 2         # score psum tiles in flight (8-bank PSUM budget)
LEAD = 3             # attention software-pipeline depth
WARMUP = 34          # PE clock-ramp dummy matmuls before the first real one
CONSUME_AV_FIRST = False  # emit AV matmuls before sums in the consume step
EXTRAS_ODD = False   # place interleaved projections at odd attention indices
COPY_DVE = False     # phase-3 copies all on DVE
PT_EXTRA = 0         # extra pm/pT ring slots beyond the exact LEAD depth
RS_EXTRA = 0         # extra sadd2/rs ring slots
WO_BUFS = 4          # wo weight tiles resident (2 per nn block-row)
RB_PAD = 0           # ropebuf padding (SBUF placement perturbation probe)
DYN_LEAD = False     # pre-produce pairs 6-9 during the last extras steps
MASK_DVE_ENTRY = False  # stretch-entry masks on DVE to smooth Pool lag
NN0_ACT = False      # first wo block row's copies on ACT (DVE drains attn)
PSO_BUFS = 2         # wo psum blocks in flight (8 banks free in phase 3)
SA_BUFS = 3          # paired avT banks in flight
REDUCE_GROUPS = 3    # attention groups using the GpSimd-reduce sums path


def _emit(tc, nc, t, out):
    persist_cm = tc.tile_pool(name="persist", bufs=1)
    persist = persist_cm.__enter__()

    # --- persistent SBUF tensors -------------------------------------------
    cosq = persist.tile([128, LQ], F16, tag="cosq")
    sinq = persist.tile([128, LQ], F16, tag="sinq")
    cosk = persist.tile([128, LK], F16, tag="cosk")
    sink = persist.tile([128, LK], F16, tag="sink")
    # binary mask (1=valid), transposed: maskq[j, min(qb,3), c*256+h*64+i]
    maskq = persist.tile([128, 4, 512], F16, tag="maskq")
    # ones matrix: sums matmul replicates column sums across all partitions
    ones = persist.tile([128, 128], F16, tag="ones")
    qT = persist.tile([128, NH, LQ], F16, tag="qT")       # RoPE'd qT, scaled
    kTr = persist.tile([128, NKV, LK], F16, tag="kTr")    # RoPE'd kT
    V = persist.tile([128, NVT, NKV * HD], F16, tag="V")  # natural [pos, d]
    # V shifted by 64 positions: odd 64-query blocks slice kv at offset 64
    Vb = persist.tile([128, NVT - 1, NKV * HD], F16, tag="Vb")
    attnT = persist.tile([128, NH, LQ], F16, tag="attnT")

    dma = nc.default_dma_engine

    def rope(ps_list, cosT, sinT, out_ap, pool):
        """out = ps*cos + swap(ps)*sin_signed (sin sign-folded on host).

        The rotate-half swap rides on DVE's per-operand partition base
        (in0 offset 64 vs out offset 0); the final add runs on GpSimd to
        keep DVE off the critical path.
        """
        col = 0
        for ps in ps_list:
            n = ps.shape[-1]
            tco = pool.tile([128, 512 + RB_PAD], F32, tag="rope_tc")
            tsi = pool.tile([128, 512 + RB_PAD], F32, tag="rope_ts")
            nc.vector.tensor_mul(tco[:, :n], ps, cosT[:, col:col + n])
            nc.vector.tensor_mul(tsi[0:64, :n], ps[64:128, :],
                                 sinT[0:64, col:col + n])
            nc.vector.tensor_mul(tsi[64:128, :n], ps[0:64, :],
                                 sinT[64:128, col:col + n])
            nc.gpsimd.tensor_add(out_ap[:, col:col + n],
                                 tco[:, :n], tsi[:, :n])
            col += n

    # --- phase 1 + 2 interleaved -------------------------------------------
    # All projection/attention pools share one scope: attention runs
    # group-by-group, with the q-head/K projections feeding group g+2
    # interleaved between its iterations, so the RoPE chains (DVE muls +
    # GpSimd add) of every group are hidden under later projection matmuls
    # and the PE never drains into a phase boundary.
    from contextlib import ExitStack
    wostream_cm = tc.tile_pool(name="wostream", bufs=WO_BUFS)
    wostream = wostream_cm.__enter__()
    outsb_cm = tc.tile_pool(name="outsb", bufs=4)
    outsb = outsb_cm.__enter__()
    es = ExitStack()
    ph1 = es.enter_context(tc.tile_pool(name="ph1", bufs=1))
    wstream = es.enter_context(tc.tile_pool(name="wstream", bufs=2))
    wvstream = es.enter_context(tc.tile_pool(name="wvstream", bufs=2))
    ropebuf = es.enter_context(tc.tile_pool(name="ropebuf", bufs=2))
    pTp = es.enter_context(tc.tile_pool(name="pT", bufs=LEAD + 2 + PT_EXTRA))
    psA = es.enter_context(tc.tile_pool(name="psA", bufs=2, space="PSUM"))
    # psT/psSA open later, after psV's banks release (PSUM reservations are
    # pool-scoped and static)

    if True:
        XT = ph1.tile([128, KT, LK], F16, tag="XT")
        wk_all = ph1.tile([128, KT, NKV * HD], F16, tag="wk")
        wq_p0 = ph1.tile([128, KT, 2 * HD], F16, tag="wq0")

        # psV scoped: its six banks release after the V/Vb copies, making
        # room for the attention psum pools that allocate mid-stream.
        psV_cm = tc.tile_pool(name="psV", bufs=1, space="PSUM")
        psV = psV_cm.__enter__()

        # V first: per-kt XT chunks let PE start early, masking the
        # input-DMA latency; wv streams alongside.
        psv = [psV.tile([128, NKV * HD], F32, tag=f"psv{t_}", name=f"psv{t_}")
               for t_ in range(NVT)]

        # HAM warm-up: the PE clock ramps only after ~3us of sustained
        # activity; tiny fp32 dummy matmuls on a zeroed scratch fill the
        # initial DMA wait so the real work starts at speed. They scribble
        # on a corner of psv[0], which the real kt=0 start=True clears.
        warm = ph1.tile([128, 16], F32, tag="warm")
        nc.vector.memset(warm, 0.0)
        for _ in range(WARMUP):
            nc.tensor.matmul(psv[0][:16, :16], lhsT=warm, rhs=warm,
                             start=True, stop=True)
        # q head 0 rides along: its matmuls fill the PE gaps while the
        # v phase is DMA-paced, and it pins only ONE psA slot, so the
        # k projections that follow start without waiting for its RoPE.
        qps0 = psA.tile([128, LQ], F32, tag="ps", name="qps0")
        for kt2 in range(KT // 2):
            if kt2 == 0:
                for kt0 in (0, 1):
                    dma.dma_start(
                        out=XT[:, kt0, :],
                        in_=t["xT"][kt0 * 128:(kt0 + 1) * 128, :])
            else:
                dma.dma_start(
                    out=XT[:, 2 * kt2:2 * kt2 + 2, :],
                    in_=t["xT"][kt2 * 256:(kt2 + 1) * 256, :]
                    .rearrange("(kt p) n -> p kt n", p=128))
            wv_2 = wvstream.tile([128, 2, NKV * HD], F16, tag="wv")
            if kt2 == 0:
                for kt0 in (0, 1):
                    dma.dma_start(
                        out=wv_2[:, kt0, :],
                        in_=t["wvT"][kt0 * 128:(kt0 + 1) * 128, :])
            else:
                dma.dma_start(
                    out=wv_2,
                    in_=t["wvT"][kt2 * 256:(kt2 + 1) * 256, :]
                    .rearrange("(kt p) d -> p kt d", p=128))
            # staged weight/table DMAs, one insert per iteration to keep
            # each XT chunk's queue wait bounded
            if kt2 < 2:
                dma.dma_start(
                    out=wq_p0[:, kt2 * 8:(kt2 + 1) * 8, :],
                    in_=t["wqT"][kt2 * 1024:(kt2 + 1) * 1024, 0:2 * HD]
                    .rearrange("(kt p) d -> p kt d", p=128))
            elif kt2 == 2:
                dma.dma_start(out=cosq, in_=t["cos_q"][:])
                dma.dma_start(out=sinq, in_=t["sin_q"][:])
            elif kt2 == 3:
                dma.dma_start(out=cosk, in_=t["cos_k"][:])
                dma.dma_start(out=sink, in_=t["sin_k"][:])
            else:
                r = kt2 - 4
                dma.dma_start(
                    out=wk_all[:, r * 4:(r + 1) * 4, :],
                    in_=t["wkT"][r * 512:(r + 1) * 512, :]
                    .rearrange("(kt p) d -> p kt d", p=128))
            for kt in (2 * kt2, 2 * kt2 + 1):
                for t_ in range(NVT):
                    pl = min(128, LK - t_ * 128)
                    nc.tensor.matmul(
                        psv[t_][:pl, :],
                        lhsT=XT[:, kt, t_ * 128:t_ * 128 + pl],
                        rhs=wv_2[:, kt - 2 * kt2, :],
                        start=(kt == 0), stop=(kt == KT - 1))
                nc.tensor.matmul(
                    qps0, lhsT=wq_p0[:, kt, 0:HD],
                    rhs=XT[:, kt, HALO:],
                    start=(kt == 0), stop=(kt == KT - 1))
        # attention-only tables queue behind the loop
        dma.dma_start(out=maskq,
                      in_=t["maskq"][:].rearrange("p (m q) -> p m q", m=4))
        dma.dma_start(out=ones, in_=t["ones"][:])
        rope([qps0], cosq, sinq, qT[:, 0, :], ropebuf)

        # V/Vb copies ride the otherwise-idle ACT engine so the k0 RoPE
        # (which releases the k psum slots) isn't queued behind them on DVE.
        for t_ in range(NVT):
            pl = min(128, LK - t_ * 128)
            nc.scalar.copy(V[:pl, t_, :], psv[t_][:pl, :])
        for t_ in range(NVT - 1):
            nc.scalar.copy(Vb[0:64, t_, :], psv[t_][64:128, :])
            pl = min(64, LK - (t_ + 1) * 128)
            nc.scalar.copy(Vb[64:64 + pl, t_, :], psv[t_ + 1][:pl, :])
        psV_cm.__exit__(None, None, None)
        psT = es.enter_context(
            tc.tile_pool(name="psT", bufs=PST_BUFS, space="PSUM"))
        # paired avT banks (two iterations per bank)
        psSAa = es.enter_context(
            tc.tile_pool(name="psSAa", bufs=SA_BUFS, space="PSUM"))
        rsump = es.enter_context(
            tc.tile_pool(name="rsump", bufs=LEAD // 2 + 2 + RS_EXTRA))

        # wq^T streamed two heads per DMA so the innermost contiguous run
        # stays >= 512B (full fp16 DMA throughput).
        wq_pair = {}

        def load_wq_pair(p):
            w = wstream.tile([128, KT, 2 * HD], F16, tag="w")
            dma.dma_start(
                out=w,
                in_=t["wqT"][:, 2 * p * HD:(2 * p + 2) * HD]
                .rearrange("(kt pp) d -> pp kt d", pp=128))
            wq_pair[p] = w

        def emit_qhead(h):
            p_idx = h // 2
            if h % 2 == 0 and p_idx + 1 < NH // 2:
                load_wq_pair(p_idx + 1)
            w = wq_p0 if p_idx == 0 else wq_pair[p_idx]
            ps = psA.tile([128, LQ], F32, tag="ps")
            for kt in range(KT):
                nc.tensor.matmul(
                    ps, lhsT=w[:, kt, (h % 2) * HD:(h % 2 + 1) * HD],
                    rhs=XT[:, kt, HALO:],
                    start=(kt == 0), stop=(kt == KT - 1))
            rope([ps], cosq, sinq, qT[:, h, :], ropebuf)

        def emit_kgroup(g):
            # K projection for one kv group; psum split 352+352.
            ps0 = psA.tile([128, 352], F32, tag="ps")
            ps1 = psA.tile([128, 352], F32, tag="ps")
            for kt in range(KT):
                nc.tensor.matmul(
                    ps0, lhsT=wk_all[:, kt, g * HD:(g + 1) * HD],
                    rhs=XT[:, kt, 0:352],
                    start=(kt == 0), stop=(kt == KT - 1))
            for kt in range(KT):
                nc.tensor.matmul(
                    ps1, lhsT=wk_all[:, kt, g * HD:(g + 1) * HD],
                    rhs=XT[:, kt, 352:LK],
                    start=(kt == 0), stop=(kt == KT - 1))
            rope([ps0, ps1], cosk, sink, kTr[:, g, :], ropebuf)

        wo_tiles = {}

        def load_wo(nn):
            halves = []
            for hh in range(2):
                w = wostream.tile([128, KT // 2, 512], F16, tag="wo",
                                  name="wo_nn")
                dma.dma_start(
                    out=w,
                    in_=t["woT"][hh * (DIM // 2):(hh + 1) * (DIM // 2),
                                 nn * 512:(nn + 1) * 512]
                    .rearrange("(ht p) n -> p ht n", p=128))
                halves.append(w)
            wo_tiles[nn] = halves

        # pre-attention projections: q heads 1..7 (head 0 rode the V phase)
        # and the first two kv groups' K, so attention groups 0-1 are ready.
        emit_qhead(1)
        emit_kgroup(0)
        load_wq_pair(1)
        emit_qhead(2)
        emit_qhead(3)
        emit_kgroup(1)
        emit_qhead(4)
        emit_qhead(5)
        emit_qhead(6)
        emit_qhead(7)
        load_wo(0)
        load_wo(1)
        if WO_BUFS >= 8:
            load_wo(2)
            load_wo(3)

        # --- attention, group-major, interleaved with remaining projections
        # Software-pipelined LEAD deep: scores/exp/mask for iteration i are
        # emitted alongside sums/AV/divide for iteration i-LEAD, so the PE's
        # in-order queue never blocks on the ACT exp -> DVE mask chain. The
        # projections feeding attention group g+2 ride between group g's
        # iterations, staying well ahead of their consumers.
        if EXTRAS_ODD:
            extras = {1: lambda: emit_qhead(8), 3: lambda: emit_qhead(9),
                      5: lambda: emit_qhead(10), 6: lambda: emit_qhead(11),
                      7: lambda: emit_kgroup(2),
                      9: lambda: emit_qhead(12), 11: lambda: emit_qhead(13),
                      13: lambda: emit_qhead(14), 14: lambda: emit_qhead(15),
                      15: lambda: emit_kgroup(3)}
        else:
            extras = {0: lambda: emit_qhead(8), 2: lambda: emit_qhead(9),
                      4: lambda: emit_qhead(10), 6: lambda: emit_qhead(11),
                      7: lambda: emit_kgroup(2),
                      8: lambda: emit_qhead(12), 10: lambda: emit_qhead(13),
                      12: lambda: emit_qhead(14), 14: lambda: emit_qhead(15),
                      15: lambda: emit_kgroup(3)}
        iters = [(qb, g) for g in range(NKV) for qb in range(2 * NQB)]
        pending = {}
        rs_store = {}
        # dynamic lead: pre-produce pairs 6-9 (iters 12-19) during steps
        # 12-15 so the pure-attention stretch starts with a deeper pm/rs
        # buffer; pairs stay adjacent in the produce sequence (the sadd2/rs
        # pair tiles fill half-by-half)
        produce_at = {}
        if DYN_LEAD:
            pair_step = {6: 12, 8: 13, 7: 14, 9: 15}
            for it_ in range(len(iters)):
                pr = it_ // 2
                if pr < 6:
                    produce_at.setdefault(it_, []).append(it_)
                elif pr in pair_step:
                    produce_at.setdefault(pair_step[pr], []).append(it_)
                else:
                    produce_at.setdefault(it_ - 4, []).append(it_)
        else:
            for it_ in range(len(iters)):
                produce_at.setdefault(it_, []).append(it_)
        for it in range(len(iters) + LEAD):
            for pit in produce_at.get(it, []):
                qb, g = iters[pit]
                q0 = qb * 64
                # moving operand: 4 heads of group g, 64-query block qb
                q_ap = qT[:, GRP * g:GRP * (g + 1), q0:q0 + 64]
                sT = psT.tile([128, 512], F32, tag="sT")
                nc.tensor.matmul(
                    sT[:, 0:256],
                    lhsT=kTr[:, g, q0:q0 + 128],
                    rhs=q_ap, start=True, stop=True)
                nc.tensor.matmul(
                    sT[:, 256:512],
                    lhsT=kTr[:, g, q0 + 128:q0 + 256],
                    rhs=q_ap, start=True, stop=True)
                pT = pTp.tile([128, 512], F16, tag="pT")
                nc.scalar.activation(
                    pT, sT, mybir.ActivationFunctionType.Exp)
                pm = pTp.tile([128, 512], F16, tag="pm")
                # binary mask on GpSimd (the SBUF-only engine); the first
                # pure-stretch iterations' masks go to DVE instead, smoothing
                # the Pool-lag transient at the stretch entry
                if MASK_DVE_ENTRY and pit in (16, 17, 18):
                    nc.vector.tensor_mul(pm, pT, maskq[:, min(qb, 3), :])
                else:
                    nc.gpsimd.tensor_mul(pm, pT, maskq[:, min(qb, 3), :])
                # produce-side softmax denominators: partition_all_reduce on
                # GpSimd gives per-column kv-sums broadcast across partitions
                # in SBUF; a 2x-mode fp16 DVE add folds the two kv chunks and
                # the all-SBUF fp16 reciprocal finishes 1/sums per pair.
                # Running this LEAD iterations ahead of the consume hides the
                # whole chain under interleaved projection matmuls, so the PE
                # never runs a sums matmul and the attention tail is AV+mul.
                sred = pTp.tile([128, 512], F16, tag="sred")
                nc.gpsimd.partition_all_reduce(
                    sred, pm, channels=128,
                    reduce_op=bass_isa.ReduceOp.add)
                if pit % 2 == 0:
                    sadd2 = rsump.tile([128, 512], F16, tag="sadd2")
                nc.vector.tensor_add(
                    sadd2[:, (pit % 2) * 256:(pit % 2) * 256 + 256],
                    sred[:, 0:256], sred[:, 256:512])
                if pit % 2 == 1:
                    rs = rsump.tile([128, 512], F16, tag="rs")
                    with nc.allow_low_precision(
                            reason="fp16 1/sums: 5e-4 rel, budget 2e-2"):
                        nc.vector.reciprocal(rs, sadd2)
                    rs_store[pit // 2] = rs
                pending[pit] = pm
            if it in extras:
                extras[it]()
            if it >= LEAD:
                mc = it - LEAD
                qb, g = iters[mc]
                half = (mc % 2) * 256
                pm = pending.pop(mc)
                if mc % 2 == 0:
                    sa_a = psSAa.tile([128, 512], F32, tag="aa", name="aa")
                for c in range(2):
                    vsrc = (V[:, qb // 2 + c, :] if qb % 2 == 0
                            else Vb[:, qb // 2 + c, :])
                    nc.tensor.matmul(
                        sa_a[:, half:half + 256],
                        lhsT=vsrc[:, g * HD:(g + 1) * HD],
                        rhs=pm[:, c * 256:(c + 1) * 256],
                        start=(c == 0), stop=(c == 1))
                if mc % 2 == 1:
                    q0 = (qb - 1) * 64
                    rs = rs_store.pop(mc // 2)
                    nc.vector.tensor_mul(
                        attnT[:, GRP * g:GRP * (g + 1), q0:q0 + 128]
                        .rearrange("p h (i q) -> p i h q", i=2),
                        sa_a.rearrange("p (i h q) -> p i h q", i=2, h=GRP),
                        rs.rearrange("p (i h q) -> p i h q", i=2, h=GRP))

    es.close()

    # --- phase 3: output projection ----------------------------------------
    if True:
        with tc.tile_pool(name="psO", bufs=PSO_BUFS, space="PSUM") as psO:
            for nn in range(4):
                if nn not in wo_tiles:
                    load_wo(nn)
                wo_nn = wo_tiles[nn]
                for pb in range(NQB):
                    if nn == 3 and pb == NQB - 1 and TAIL_NARROW:
                        # last block as 384+128 columns: the final copy+DMA
                        # shrink to a 128-wide slice, trimming the tail
                        for c0, cw in ((0, 384), (384, 128)):
                            ps = psO.tile([128, cw], F32, tag=f"psN{cw}",
                                          name="psN")
                            for ht in range(KT):
                                nc.tensor.matmul(
                                    ps,
                                    lhsT=attnT[:, ht,
                                               pb * 128:(pb + 1) * 128],
                                    rhs=wo_nn[ht // (KT // 2)]
                                    [:, ht % (KT // 2), c0:c0 + cw],
                                    start=(ht == 0), stop=(ht == KT - 1))
                            ob = outsb.tile([128, cw], F16, tag=f"obN{cw}",
                                            name="obN")
                            nc.scalar.copy(ob, ps)
                            dma.dma_start(
                                out=out[pb * 128:(pb + 1) * 128,
                                        nn * 512 + c0:nn * 512 + c0 + cw],
                                in_=ob)
                        continue
                    if nn == 3 and pb == NQB - 1 and TAIL_SPLIT:
                        # final block in TAIL_SPLIT column slices (independent
                        # accumulation groups in one psum tile): early slices'
                        # copy+DMA overlap later slices' matmuls -> short tail
                        nsl = int(TAIL_SPLIT)
                        sw = 512 // nsl
                        ps = psO.tile([128, 512], F32, tag="psO")
                        for sl in range(nsl):
                            for ht in range(KT):
                                nc.tensor.matmul(
                                    ps[:, sl * sw:(sl + 1) * sw],
                                    lhsT=attnT[:, ht,
                                               pb * 128:(pb + 1) * 128],
                                    rhs=wo_nn[ht // (KT // 2)]
                                    [:, ht % (KT // 2), sl * sw:(sl + 1) * sw],
                                    start=(ht == 0), stop=(ht == KT - 1))
                            ob = outsb.tile([128, 512], F16, tag="ob")
                            nc.scalar.copy(ob[:, 0:sw],
                                           ps[:, sl * sw:(sl + 1) * sw])
                            dma.dma_start(
                                out=out[pb * 128:(pb + 1) * 128,
                                        nn * 512 + sl * sw:
                                        nn * 512 + (sl + 1) * sw],
                                in_=ob[:, 0:sw])
                        continue
                    ps = psO.tile([128, 512], F32, tag="psO")
                    for ht in range(KT):
                        nc.tensor.matmul(
                            ps,
                            lhsT=attnT[:, ht, pb * 128:(pb + 1) * 128],
                            rhs=wo_nn[ht // (KT // 2)][:, ht % (KT // 2), :],
                            start=(ht == 0), stop=(ht == KT - 1))
                    ob = outsb.tile([128, 512], F16, tag="ob")
                    if (nn == 3 or (nn == 0 and NN0_ACT)) \
                            and not COPY_DVE:
                        nc.scalar.copy(ob, ps)   # ACT: keep DVE off the tail
                    else:
                        nc.vector.tensor_copy(ob, ps)
                    dma.dma_start(
                        out=out[pb * 128:(pb + 1) * 128,
                                nn * 512:(nn + 1) * 512],
                        in_=ob)

    outsb_cm.__exit__(None, None, None)
    wostream_cm.__exit__(None, None, None)
    persist_cm.__exit__(None, None, None)


def _build_nc():
    nc = bacc.Bacc()
    specs = {
        "xT": ([DIM, LK], F16),
        "cos_q": ([128, LQ], F16), "sin_q": ([128, LQ], F16),
        "cos_k": ([128, LK], F16), "sin_k": ([128, LK], F16),
        "maskq": ([128, 4 * 512], F16),
        "wqT": ([DIM, NH * HD], F16), "wkT": ([DIM, NKV * HD], F16),
        "wvT": ([DIM, NKV * HD], F16), "woT": ([NH * HD, DIM], F16),
        "ones": ([128, 128], F16),
    }
    t = {n: nc.declare_dram_parameter(n, s, d, isOutput=False)
         for n, (s, d) in specs.items()}
    out = nc.declare_dram_parameter("out", [LQ, DIM], F16, isOutput=True)
    with tile.TileContext(nc) as tc:
        _emit(tc, nc, t, out)
    nc.finalize()
    return nc


def _core_inputs(xT_full, cos, sin, wqT, wkT, wvT, woT, core):
    b, chunk = core // 4, core % 4
    g0 = chunk * LQ
    lo = g0 - HALO

    xT = np.zeros((DIM, LK), np.float16)
    src_lo = max(lo, 0)
    xT[:, src_lo - lo:] = xT_full[b][:, src_lo:g0 + LQ]

    kpos = np.clip(np.arange(lo, g0 + LQ), 0, None)
    qpos = np.arange(g0, g0 + LQ)
    sgn = np.concatenate(
        [-np.ones(HD // 2), np.ones(HD // 2)]).astype(np.float32)

    # binary validity mask (1=keep), transposed for the post-exp multiply:
    # maskq[j, idx, c*256 + h*64 + i]; idx = min(qb, 3) selects the pattern
    # (qb >= 3 patterns are identical; qb < 3 differ only on halo-clamped
    # cores where kv positions fall off the sequence start)
    maskq = np.zeros((128, 4, 512), np.float16)
    for idx in range(4):
        qb = idx
        for c in range(2):
            j = qb * 64 + c * 128 + np.arange(128)[:, None]   # kv halo pos
            i = np.arange(64)[None, :]                        # q local pos
            d = (g0 + qb * 64 + i) - (lo + j)
            valid = (d >= 0) & (d <= W) & ((lo + j) >= 0)
            maskq[:, idx, c * 256:(c + 1) * 256] = np.tile(
                valid.astype(np.float16), (1, GRP))

    return {
        "xT": xT,
        "cos_q": np.ascontiguousarray(
            (cos[qpos] * SCALE).T.astype(np.float16)),
        "sin_q": np.ascontiguousarray(
            (sin[qpos] * sgn * SCALE).T.astype(np.float16)),
        "cos_k": np.ascontiguousarray(cos[kpos].T.astype(np.float16)),
        "sin_k": np.ascontiguousarray((sin[kpos] * sgn).T.astype(np.float16)),
        "maskq": np.ascontiguousarray(maskq.reshape(128, 4 * 512)),
        "ones": np.ones((128, 128), np.float16),
        "wqT": wqT.astype(np.float16), "wkT": wkT.astype(np.float16),
        "wvT": wvT.astype(np.float16), "woT": woT.astype(np.float16),
    }


def _build_runner(nc, n_cores=8):
    """jit the SPMD body once so repeat kernel() calls skip retracing."""
    import jax
    from jax.experimental.shard_map import shard_map
    from jax.sharding import Mesh, NamedSharding, PartitionSpec

    from concourse import bass2jax

    bass2jax.install_neuronx_cc_hook()
    partition_name = (nc.partition_id_tensor.name
                      if nc.partition_id_tensor else None)
    in_names, out_names, out_avals = [], [], []
    for alloc in nc.m.functions[0].allocations:
        if not isinstance(alloc, mybir.MemoryLocationSet):
            continue
        name = alloc.memorylocations[0].name
        if alloc.kind == "ExternalInput":
            if name != partition_name:
                in_names.append(name)
        elif alloc.kind == "ExternalOutput":
            out_names.append(name)
            out_avals.append(jax.core.ShapedArray(
                tuple(alloc.tensor_shape), mybir.dt.np(alloc.dtype)))
    all_in = list(in_names) + list(out_names)
    if partition_name is not None:
        all_in.append(partition_name)

    def _body(*args):
        operands = list(args)
        if partition_name is not None:
            operands.append(bass2jax.partition_id_tensor())
        return tuple(bass2jax._bass_exec_p.bind(
            *operands, out_avals=tuple(out_avals), in_names=tuple(all_in),
            out_names=tuple(out_names), lowering_input_output_aliases=(),
            sim_require_finite=True, sim_require_nnan=True, nc=nc))

    devices = jax.devices()[:n_cores]
    mesh = Mesh(np.asarray(devices), ("core",))
    nspec = (PartitionSpec("core"),)
    sharded = jax.jit(
        shard_map(_body, mesh=mesh,
                  in_specs=nspec * (len(in_names) + len(out_avals)),
                  out_specs=nspec * len(out_avals), check_rep=False),
        keep_unused=True)
    sharding = NamedSharding(mesh, PartitionSpec("core"))
    zeros = [jax.device_put(
        np.zeros((n_cores * a.shape[0], *a.shape[1:]), a.dtype), sharding)
        for a in out_avals]
    return {"fn": sharded, "in_names": in_names, "out_names": out_names,
            "out_avals": out_avals, "sharding": sharding, "zeros": zeros,
            "dev_cache": {}}


def _run_cached(runner, in_maps):
    """Repeat-call path: device-cache replicated tensors by fingerprint."""
    import hashlib

    import jax

    n_cores = len(in_maps)
    args = []
    for name in runner["in_names"]:
        arrs = [np.asarray(in_maps[c][name]) for c in range(n_cores)]
        replicated = all(a is arrs[0] or np.shares_memory(a, arrs[0])
                         for a in arrs)
        if replicated:
            h = hashlib.blake2b(arrs[0].tobytes(), digest_size=16).hexdigest()
            key = (name, h)
            if key not in runner["dev_cache"]:
                runner["dev_cache"] = {k: v for k, v in
                                       runner["dev_cache"].items()
                                       if k[0] != name}
                runner["dev_cache"][key] = jax.device_put(
                    np.concatenate(arrs, axis=0), runner["sharding"])
            args.append(runner["dev_cache"][key])
        else:
            args.append(jax.device_put(np.concatenate(arrs, axis=0),
                                       runner["sharding"]))
    outs = runner["fn"](*args, *runner["zeros"])
    outs = [np.asarray(o) for o in outs]
    return [{name: outs[i].reshape(n_cores, *runner["out_avals"][i].shape)[c]
             for i, name in enumerate(runner["out_names"])}
            for c in range(n_cores)]


def kernel(x, cos, sin, wq, wk, wv, wo, _return_results=False):
    x = np.ascontiguousarray(np.asarray(x, np.float32))
    cos = np.asarray(cos, np.float32)
    sin = np.asarray(sin, np.float32)
    wqT = np.ascontiguousarray(np.asarray(wq, np.float32).T)
    wkT = np.ascontiguousarray(np.asarray(wk, np.float32).T)
    wvT = np.ascontiguousarray(np.asarray(wv, np.float32).T)
    woT = np.ascontiguousarray(np.asarray(wo, np.float32).T)

    if "nc" not in _CACHE:
        _CACHE["nc"] = _build_nc()
    nc = _CACHE["nc"]

    xT_full = np.ascontiguousarray(x.transpose(0, 2, 1))
    in_maps = [_core_inputs(xT_full, cos, sin, wqT, wkT, wvT, woT, core)
               for core in range(8)]

    res = None
    if not _CACHE.get("ran_once"):
        # first call: the documented run_bass_kernel_spmd path (compiles
        # the NEFF); later calls reuse a cached jitted runner.
        res = run_bass_kernel_spmd(nc, in_maps, core_ids=list(range(8)))
        results = res.results
        _CACHE["ran_once"] = True
    else:
        if "runner" not in _CACHE:
            try:
                _CACHE["runner"] = _build_runner(nc)
            except Exception:
                _CACHE["runner"] = None
        if _CACHE["runner"] is not None:
            results = _run_cached(_CACHE["runner"], in_maps)
        else:
            res = run_bass_kernel_spmd(nc, in_maps, core_ids=list(range(8)))
            results = res.results

    full = np.zeros((B, L, DIM), np.float32)
    for core in range(8):
        b, chunk = core // 4, core % 4
        full[b, chunk * LQ:(chunk + 1) * LQ] = results[core]["out"]
    if _return_results:
        return full, res
    return full



# revision 3
# speedup vs baseline: 1.1836x; 1.1836x over previous
"""Sliding-window GQA attention block (RoPE + QKV proj + SWA + out proj) on 8
Trainium2 NeuronCores.

Sharding: batch (2) x sequence chunks (4 x 512) -> 8 cores, SPMD. Each core
computes a 512-query slice of the output using a 192-position K/V halo, so no
cross-core reduction is needed; outputs concatenate exactly.

Per-core dataflow: the four projection matmuls (Q, K, V, out) run as fp8e4m3
DoubleRow matmuls -- each instruction contracts TWO 128-row k-tiles at 0.5
cycles/row, 4x fp16 throughput per output column. Precision is recovered with
a hi/lo split of both operands (w = wh + wl, x = xh + xl, quantized at scales
64/8); per k-tile pair three DoubleRow matmuls accumulate wh*xh + wl*xh +
wh*xl (the dropped l*l term is ~0.1% of the output), so the projections cost
3/4 of their fp16 column count at ~1.5e-3 final absmax rel err. The psum
carries a 512x scale that folds for free into the RoPE table multiply (Q/K),
the exp's input scale (scores), and the ACT copy scale (V, output). Each
DoubleRow chunk is 256 columns (the 512-element moving-operand limit covers
both planes), and each PSUM bank runs ONE accumulation group: start on the
first matmul into the bank, stop on the last, chunks interleaved freely
in between (bank-granular pending-zero semantics).

Attention itself stays fp16, identical to the fp16 baseline: 64-query blocks,
two 128-row kv chunks per 257-wide window into one [128,512] PSUM bank, one
Exp per block (input scale folds the q/k psum scales), binary mask multiply
on GpSimd, softmax sums via GpSimd partition_all_reduce, fp16 reciprocal +
paired DVE normalization. V tiles hold 16*v so attnT comes out pre-scaled
for its own fp8 hi/lo split (ACT copy hi, GpSimd subtract lo), feeding the
DoubleRow out-projection.

All weight tensors are host-packed into partition-contiguous tile images so
every DMA moves >=512B contiguous runs at full bus rate; fp8 hi+lo totals the
same bytes as the fp16 baseline. The q-head/K projections feeding attention
group g+2 ride between group g's iterations as before, so the PE queue never
drains into a phase boundary; q head 0 rides the V phase two pairs behind
the x/wv streams.
"""
import numpy as np
import ml_dtypes

import concourse.tile as tile
from concourse import bacc, bass_isa, mybir
from concourse.bass_utils import run_bass_kernel_spmd

F32 = mybir.dt.float32
F16 = mybir.dt.float16
F8 = mybir.dt.float8e4
NPF8 = ml_dtypes.float8_e4m3
DR = mybir.MatmulPerfMode.DoubleRow

B, L, DIM = 2, 2048, 2048
NH, NKV, HD, W = 16, 4, 128, 192
LQ, HALO = 512, 192
LK = LQ + HALO              # 704
KT = DIM // 128             # 16 contraction tiles
NJ = KT // 2                # 8 k-tile pairs (DoubleRow planes)
NQB = LQ // 128             # 4 query blocks
NVT = (LK + 127) // 128     # 6 value pos-tiles
SCALE = HD ** -0.5
GRP = NH // NKV             # 4 query heads per kv head

SX = 8.0                    # x quantization scale
SW = 64.0                   # weight quantization scale
SA = 16.0                   # attnT scale (folded into the V copy scale)
PSCALE = SX * SW            # 512: scale carried by projection psums
# (w_plane, x_plane): (hi,hi) main term, (lo,hi) and (hi,lo) corrections
TERMS = ((0, 0), (1, 0), (0, 1))

_CACHE = {}

# tuning knobs
PST_BUFS = 2         # score psum tiles in flight (8-bank PSUM budget)
LEAD = 3             # attention software-pipeline depth
WARMUP = 34          # PE clock-ramp dummy matmuls before the first real one
WO_BUFS = 2          # wo weight images resident (1 per nn block)
SA_BUFS = 3          # paired avT banks in flight
PSO_BUFS = 2         # wo psum blocks in flight
QLAG = 2             # V-phase q-head-0 ride lags the x/wv stream by 2 pairs


def _emit(tc, nc, t, out):
    persist_cm = tc.tile_pool(name="persist", bufs=1)
    persist = persist_cm.__enter__()

    # --- persistent SBUF tensors -------------------------------------------
    cosq = persist.tile([128, LQ], F16, tag="cosq")
    sinq = persist.tile([128, LQ], F16, tag="sinq")
    cosk = persist.tile([128, LK], F16, tag="cosk")
    sink = persist.tile([128, LK], F16, tag="sink")
    maskq = persist.tile([128, 4, 512], F16, tag="maskq")
    qT = persist.tile([128, NH, LQ], F16, tag="qT")       # RoPE'd qT, scaled
    kTr = persist.tile([128, NKV, LK], F16, tag="kTr")    # RoPE'd kT
    V = persist.tile([128, NVT, NKV * HD], F16, tag="V")  # holds SA*v
    Vb = persist.tile([128, NVT - 1, NKV * HD], F16, tag="Vb")
    attnT = persist.tile([128, NH, LQ], F16, tag="attnT")  # SA*attn fp16
    attnH = persist.tile([128, NH, LQ], F8, tag="attnH")   # hi fp8 plane
    attnL = persist.tile([128, NH, LQ], F8, tag="attnL")   # lo fp8 plane
    # x planes, pair-interleaved for DoubleRow: [p, pair, plane, pos]
    XTh = persist.tile([128, NJ, 2, LK], F8, tag="XTh")
    XTl = persist.tile([128, NJ, 2, LK], F8, tag="XTl")

    dma = nc.default_dma_engine

    def rope(ps_list, cosT, sinT, out_ap, pool):
        """out = ps*cos + swap(ps)*sin_signed (sin sign-folded on host).

        ps carries the PSCALE projection scale; the tables are sized so
        out_ap lands at its designed fp16 scale."""
        col = 0
        for ps in ps_list:
            n = ps.shape[-1]
            tco = pool.tile([128, 512], F32, tag="rope_tc")
            tsi = pool.tile([128, 512], F32, tag="rope_ts")
            nc.vector.tensor_mul(tco[:, :n], ps, cosT[:, col:col + n])
            nc.vector.tensor_mul(tsi[0:64, :n], ps[64:128, :],
                                 sinT[0:64, col:col + n])
            nc.vector.tensor_mul(tsi[64:128, :n], ps[0:64, :],
                                 sinT[64:128, col:col + n])
            nc.gpsimd.tensor_add(out_ap[:, col:col + n],
                                 tco[:, :n], tsi[:, :n])
            col += n

    # --- phase 1 + 2 interleaved -------------------------------------------
    from contextlib import ExitStack
    wostream_cm = tc.tile_pool(name="wostream", bufs=WO_BUFS)
    wostream = wostream_cm.__enter__()
    outsb_cm = tc.tile_pool(name="outsb", bufs=4)
    outsb = outsb_cm.__enter__()
    es = ExitStack()
    ph1 = es.enter_context(tc.tile_pool(name="ph1", bufs=1))
    wstream = es.enter_context(tc.tile_pool(name="wstream", bufs=2))
    wvstream = es.enter_context(tc.tile_pool(name="wvstream", bufs=2))
    ropebuf = es.enter_context(tc.tile_pool(name="ropebuf", bufs=2))
    pTp = es.enter_context(tc.tile_pool(name="pT", bufs=LEAD + 2))
    psA = es.enter_context(tc.tile_pool(name="psA", bufs=2, space="PSUM"))

    if True:
        # wk image: [p, pair, plane, hl, g*128+m]
        wk8 = ph1.tile([128, NJ, 2, 2, NKV * HD], F8, tag="wk8")

        psV_cm = tc.tile_pool(name="psV", bufs=1, space="PSUM")
        psV = psV_cm.__enter__()
        psv = [psV.tile([128, NKV * HD], F32, tag=f"psv{t_}", name=f"psv{t_}")
               for t_ in range(NVT)]

        # HAM warm-up: fill the initial DMA wait so the PE clock ramps.
        warm = ph1.tile([128, 16], F32, tag="warm")
        nc.vector.memset(warm, 0.0)
        for _ in range(WARMUP):
            nc.tensor.matmul(psv[0][:16, :16], lhsT=warm, rhs=warm,
                             start=True, stop=True)

        wq_pair = {}

        def load_wq_pair(p):
            w = wstream.tile([128, NJ, 2, 2, 256], F8, tag="w")
            dma.dma_start(
                out=w,
                in_=t["wq8"][:, p * 8192:(p + 1) * 8192]
                .rearrange("p (j i hl m) -> p j i hl m", j=NJ, i=2, hl=2))
            wq_pair[p] = w

        def load_x(j):
            dma.dma_start(
                out=XTh[:, j, :, :],
                in_=t["xTh"][j * 256:(j + 1) * 256, :]
                .rearrange("(i p) n -> p i n", p=128))
            dma.dma_start(
                out=XTl[:, j, :, :],
                in_=t["xTl"][j * 256:(j + 1) * 256, :]
                .rearrange("(i p) n -> p i n", p=128))

        wv_tiles = {}

        def load_wv(j):
            w = wvstream.tile([128, 2, 2, NKV * HD], F8, tag="wv")
            dma.dma_start(
                out=w,
                in_=t["wv8"][:, j * 2048:(j + 1) * 2048]
                .rearrange("p (i hl d) -> p i hl d", i=2, hl=2))
            wv_tiles[j] = w

        def q_mms(h, w, ps, j):
            m0 = (h % 2) * 128
            for ti, (wl, xl) in enumerate(TERMS):
                wap = w[:, j, :, wl, m0:m0 + 128]
                xt = XTl if xl else XTh
                for c in range(2):
                    nc.tensor.matmul(
                        ps[:, c * 256:(c + 1) * 256],
                        lhsT=wap,
                        rhs=xt[:, j, :, HALO + c * 256:HALO + (c + 1) * 256],
                        start=(j == 0 and ti == 0 and c == 0),
                        stop=(j == NJ - 1 and ti == 2 and c == 1),
                        perf_mode=DR)

        # V projection, q head 0 riding QLAG pairs behind the streams.
        qps0 = psA.tile([128, LQ], F32, tag="ps", name="qps0")
        load_x(0)
        load_wv(0)
        load_x(1)
        load_wv(1)
        load_wq_pair(0)
        for j in range(NJ):
            if j >= 1 and j + 1 < NJ:
                load_x(j + 1)
                load_wv(j + 1)
            if j == 1:
                dma.dma_start(out=cosq, in_=t["cos_q"][:])
                dma.dma_start(out=sinq, in_=t["sin_q"][:])
            wv_j = wv_tiles.pop(j)
            for ti, (wl, xl) in enumerate(TERMS):
                xt = XTl if xl else XTh
                for t_ in range(NVT):
                    pl = min(128, LK - t_ * 128)
                    lhs = xt[:, j, :, t_ * 128:t_ * 128 + pl]
                    for c in range(2):
                        nc.tensor.matmul(
                            psv[t_][:pl, c * 256:(c + 1) * 256],
                            lhsT=lhs,
                            rhs=wv_j[:, :, wl, c * 256:(c + 1) * 256],
                            start=(j == 0 and ti == 0 and c == 0),
                            stop=(j == NJ - 1 and ti == 2 and c == 1),
                            perf_mode=DR)
            if j >= QLAG:
                q_mms(0, wq_pair[0], qps0, j - QLAG)
        for j in range(NJ - QLAG, NJ):
            q_mms(0, wq_pair[0], qps0, j)
        rope([qps0], cosq, sinq, qT[:, 0, :], ropebuf)

        # V/Vb copies on ACT with the SA/PSCALE descale folded in.
        VS = SA / PSCALE
        for t_ in range(NVT):
            pl = min(128, LK - t_ * 128)
            nc.scalar.activation(V[:pl, t_, :], psv[t_][:pl, :],
                                 mybir.ActivationFunctionType.Copy, scale=VS)
        for t_ in range(NVT - 1):
            nc.scalar.activation(Vb[0:64, t_, :], psv[t_][64:128, :],
                                 mybir.ActivationFunctionType.Copy, scale=VS)
            pl = min(64, LK - (t_ + 1) * 128)
            nc.scalar.activation(Vb[64:64 + pl, t_, :], psv[t_ + 1][:pl, :],
                                 mybir.ActivationFunctionType.Copy, scale=VS)
        psV_cm.__exit__(None, None, None)
        psT = es.enter_context(
            tc.tile_pool(name="psT", bufs=PST_BUFS, space="PSUM"))
        psSAa = es.enter_context(
            tc.tile_pool(name="psSAa", bufs=SA_BUFS, space="PSUM"))
        rsump = es.enter_context(tc.tile_pool(name="rsump", bufs=LEAD // 2 + 2))

        def emit_qhead(h):
            p_idx = h // 2
            if h % 2 == 0 and p_idx + 1 < NH // 2:
                load_wq_pair(p_idx + 1)
            w = wq_pair[p_idx]
            ps = psA.tile([128, LQ], F32, tag="ps")
            for j in range(NJ):
                q_mms(h, w, ps, j)
            rope([ps], cosq, sinq, qT[:, h, :], ropebuf)

        def emit_kgroup(g):
            # K projection for one kv group; psum split 448 + 256 so each
            # DoubleRow chunk stays inside a PSUM bank. One accumulation
            # group per tile: start on its first chunk, stop on its last.
            ps0 = psA.tile([128, 448], F32, tag="ps")
            ps1 = psA.tile([128, 256], F32, tag="ps")
            chunks = ((ps0, 0, 0, 256, True, False), (ps0, 256, 256, 192,
                                                      False, True),
                      (ps1, 0, 448, 256, True, True))
            for j in range(NJ):
                for ti, (wl, xl) in enumerate(TERMS):
                    wap = wk8[:, j, :, wl, g * HD:(g + 1) * HD]
                    xt = XTl if xl else XTh
                    first = (j == 0 and ti == 0)
                    last = (j == NJ - 1 and ti == 2)
                    for ps, po, xo, n, c_first, c_last in chunks:
                        nc.tensor.matmul(
                            ps[:, po:po + n],
                            lhsT=wap,
                            rhs=xt[:, j, :, xo:xo + n],
                            start=(first and c_first),
                            stop=(last and c_last), perf_mode=DR)
            rope([ps0, ps1], cosk, sink, kTr[:, g, :], ropebuf)

        wo_tiles = {}

        def load_wo(nn):
            w = wostream.tile([128, NJ, 2, 2, 512], F8, tag="wo", name="wo_nn")
            dma.dma_start(
                out=w,
                in_=t["wo8"][:, nn * 16384:(nn + 1) * 16384]
                .rearrange("p (j i hl n) -> p j i hl n", j=NJ, i=2, hl=2))
            wo_tiles[nn] = w

        # pre-attention projections: q heads 1..7 and kv groups 0-1, with
        # the wk image / k tables / mask streaming between the q heads.
        emit_qhead(1)
        for r in range(2):
            dma.dma_start(
                out=wk8[:, 2 * r:2 * r + 2, :, :, :],
                in_=t["wk8"][:, r * 4096:(r + 1) * 4096]
                .rearrange("p (j i hl d) -> p j i hl d", j=2, i=2, hl=2))
        load_wq_pair(1)
        dma.dma_start(out=cosk, in_=t["cos_k"][:])
        dma.dma_start(out=sink, in_=t["sin_k"][:])
        emit_qhead(2)
        for r in range(2, 4):
            dma.dma_start(
                out=wk8[:, 2 * r:2 * r + 2, :, :, :],
                in_=t["wk8"][:, r * 4096:(r + 1) * 4096]
                .rearrange("p (j i hl d) -> p j i hl d", j=2, i=2, hl=2))
        emit_qhead(3)
        emit_kgroup(0)
        emit_qhead(4)
        emit_qhead(5)
        emit_kgroup(1)
        emit_qhead(6)
        dma.dma_start(out=maskq,
                      in_=t["maskq"][:].rearrange("p (m q) -> p m q", m=4))
        emit_qhead(7)
        load_wo(0)

        # --- attention, group-major, interleaved with remaining projections
        extras = {0: lambda: emit_qhead(8), 2: lambda: emit_qhead(9),
                  4: lambda: emit_qhead(10), 6: lambda: emit_qhead(11),
                  7: lambda: emit_kgroup(2),
                  8: lambda: emit_qhead(12), 10: lambda: emit_qhead(13),
                  12: lambda: emit_qhead(14), 14: lambda: emit_qhead(15),
                  15: lambda: emit_kgroup(3), 16: lambda: load_wo(1)}
        iters = [(qb, g) for g in range(NKV) for qb in range(2 * NQB)]
        pending = {}
        rs_store = {}
        ESCALE = 1.0 / (PSCALE * PSCALE)
        for it in range(len(iters) + LEAD):
            if it < len(iters):
                pit = it
                qb, g = iters[pit]
                q0 = qb * 64
                q_ap = qT[:, GRP * g:GRP * (g + 1), q0:q0 + 64]
                sT = psT.tile([128, 512], F32, tag="sT")
                nc.tensor.matmul(
                    sT[:, 0:256],
                    lhsT=kTr[:, g, q0:q0 + 128],
                    rhs=q_ap, start=True, stop=True)
                nc.tensor.matmul(
                    sT[:, 256:512],
                    lhsT=kTr[:, g, q0 + 128:q0 + 256],
                    rhs=q_ap, start=True, stop=True)
                pT = pTp.tile([128, 512], F16, tag="pT")
                # the 512^2 q/k psum scales fold into the exp input scale
                nc.scalar.activation(
                    pT, sT, mybir.ActivationFunctionType.Exp, scale=ESCALE)
                pm = pTp.tile([128, 512], F16, tag="pm")
                nc.gpsimd.tensor_mul(pm, pT, maskq[:, min(qb, 3), :])
                sred = pTp.tile([128, 512], F16, tag="sred")
                nc.gpsimd.partition_all_reduce(
                    sred, pm, channels=128,
                    reduce_op=bass_isa.ReduceOp.add)
                if pit % 2 == 0:
                    sadd2 = rsump.tile([128, 512], F16, tag="sadd2")
                nc.vector.tensor_add(
                    sadd2[:, (pit % 2) * 256:(pit % 2) * 256 + 256],
                    sred[:, 0:256], sred[:, 256:512])
                if pit % 2 == 1:
                    rs = rsump.tile([128, 512], F16, tag="rs")
                    with nc.allow_low_precision(
                            reason="fp16 1/sums: 5e-4 rel, budget 2e-2"):
                        nc.vector.reciprocal(rs, sadd2)
                    rs_store[pit // 2] = rs
                pending[pit] = pm
            if it in extras:
                extras[it]()
            if it >= LEAD:
                mc = it - LEAD
                qb, g = iters[mc]
                half = (mc % 2) * 256
                pm = pending.pop(mc)
                if mc % 2 == 0:
                    sa_a = psSAa.tile([128, 512], F32, tag="aa", name="aa")
                for c in range(2):
                    vsrc = (V[:, qb // 2 + c, :] if qb % 2 == 0
                            else Vb[:, qb // 2 + c, :])
                    nc.tensor.matmul(
                        sa_a[:, half:half + 256],
                        lhsT=vsrc[:, g * HD:(g + 1) * HD],
                        rhs=pm[:, c * 256:(c + 1) * 256],
                        start=(c == 0), stop=(c == 1))
                if mc % 2 == 1:
                    q0 = (qb - 1) * 64
                    rs = rs_store.pop(mc // 2)
                    aslice = attnT[:, GRP * g:GRP * (g + 1), q0:q0 + 128]
                    nc.vector.tensor_mul(
                        aslice.rearrange("p h (i q) -> p i h q", i=2),
                        sa_a.rearrange("p (i h q) -> p i h q", i=2, h=GRP),
                        rs.rearrange("p (i h q) -> p i h q", i=2, h=GRP))
                    # fp8 hi/lo split of the freshly produced attnT slice
                    hslice = attnH[:, GRP * g:GRP * (g + 1), q0:q0 + 128]
                    lslice = attnL[:, GRP * g:GRP * (g + 1), q0:q0 + 128]
                    nc.scalar.copy(hslice, aslice)
                    nc.gpsimd.tensor_sub(lslice, aslice, hslice)

    es.close()

    # --- phase 3: output projection ----------------------------------------
    OSCALE = 1.0 / (SA * SW)
    if True:
        with tc.tile_pool(name="psO", bufs=PSO_BUFS, space="PSUM") as psO:
            for nn in range(4):
                if nn not in wo_tiles:
                    load_wo(nn)
                if nn + 1 < 4 and nn + 1 not in wo_tiles:
                    load_wo(nn + 1)
                wo_nn = wo_tiles.pop(nn)
                for pb in range(NQB):
                    ps = psO.tile([128, 512], F32, tag="psO")
                    for j in range(NJ):
                        for ti, (wl, xl) in enumerate(TERMS):
                            at = attnL if xl else attnH
                            lhs = at[:, 2 * j:2 * j + 2,
                                     pb * 128:(pb + 1) * 128]
                            for c in range(2):
                                nc.tensor.matmul(
                                    ps[:, c * 256:(c + 1) * 256],
                                    lhsT=lhs,
                                    rhs=wo_nn[:, j, :, wl,
                                              c * 256:(c + 1) * 256],
                                    start=(j == 0 and ti == 0 and c == 0),
                                    stop=(j == NJ - 1 and ti == 2 and c == 1),
                                    perf_mode=DR)
                    ob = outsb.tile([128, 512], F16, tag="ob")
                    nc.scalar.activation(ob, ps,
                                         mybir.ActivationFunctionType.Copy,
                                         scale=OSCALE)
                    dma.dma_start(
                        out=out[pb * 128:(pb + 1) * 128,
                                nn * 512:(nn + 1) * 512],
                        in_=ob)

    outsb_cm.__exit__(None, None, None)
    wostream_cm.__exit__(None, None, None)
    persist_cm.__exit__(None, None, None)


def _build_nc():
    nc = bacc.Bacc()
    specs = {
        "xTh": ([DIM, LK], F8), "xTl": ([DIM, LK], F8),
        "cos_q": ([128, LQ], F16), "sin_q": ([128, LQ], F16),
        "cos_k": ([128, LK], F16), "sin_k": ([128, LK], F16),
        "maskq": ([128, 4 * 512], F16),
        "wq8": ([128, (NH // 2) * 8192], F8),
        "wk8": ([128, 16384], F8),
        "wv8": ([128, 16384], F8),
        "wo8": ([128, 4 * 16384], F8),
    }
    t = {n: nc.declare_dram_parameter(n, s, d, isOutput=False)
         for n, (s, d) in specs.items()}
    out = nc.declare_dram_parameter("out", [LQ, DIM], F16, isOutput=True)
    with tile.TileContext(nc) as tc:
        _emit(tc, nc, t, out)
    nc.finalize()
    return nc


def _q8(a):
    return a.astype(NPF8)


def _hilo(a, s):
    h = _q8(a * s)
    l = _q8(a * s - h.astype(np.float32))
    return h, l


def _pack_weights(wqT, wkT, wvT, woT):
    """Pack hi/lo fp8 weight planes into partition-contiguous DMA images.

    Row index k of each *T tensor maps to (pair, plane, p) = (k//256,
    (k//128)%2, k%128): plane i of pair j lives in partition p."""
    def img(wT, blk, nblk):
        h, l = _hilo(wT.astype(np.float32), SW)
        A = np.stack([h, l], 0)                    # [hl, 2048, ncols]
        A = A.reshape(2, NJ, 2, 128, nblk, blk)    # [hl, j, i, p, b, m]
        A = A.transpose(3, 4, 1, 2, 0, 5)          # [p, b, j, i, hl, m]
        return np.ascontiguousarray(A.reshape(128, -1))

    return {
        "wq8": img(wqT, 256, NH // 2),
        "wk8": img(wkT, NKV * HD, 1),
        "wv8": img(wvT, NKV * HD, 1),
        "wo8": img(woT, 512, 4),
    }


def _core_inputs(xTh_full, xTl_full, cos, sin, wpack, core):
    b, chunk = core // 4, core % 4
    g0 = chunk * LQ
    lo = g0 - HALO

    xTh = np.zeros((DIM, LK), NPF8)
    xTl = np.zeros((DIM, LK), NPF8)
    src_lo = max(lo, 0)
    xTh[:, src_lo - lo:] = xTh_full[b][:, src_lo:g0 + LQ]
    xTl[:, src_lo - lo:] = xTl_full[b][:, src_lo:g0 + LQ]

    kpos = np.clip(np.arange(lo, g0 + LQ), 0, None)
    qpos = np.arange(g0, g0 + LQ)
    sgn = np.concatenate(
        [-np.ones(HD // 2), np.ones(HD // 2)]).astype(np.float32)

    maskq = np.zeros((128, 4, 512), np.float16)
    for idx in range(4):
        qb = idx
        for c in range(2):
            j = qb * 64 + c * 128 + np.arange(128)[:, None]   # kv halo pos
            i = np.arange(64)[None, :]                        # q local pos
            d = (g0 + qb * 64 + i) - (lo + j)
            valid = (d >= 0) & (d <= W) & ((lo + j) >= 0)
            maskq[:, idx, c * 256:(c + 1) * 256] = np.tile(
                valid.astype(np.float16), (1, GRP))

    ci = {
        "xTh": xTh, "xTl": xTl,
        "cos_q": np.ascontiguousarray(
            (cos[qpos] * SCALE).T.astype(np.float16)),
        "sin_q": np.ascontiguousarray(
            (sin[qpos] * sgn * SCALE).T.astype(np.float16)),
        "cos_k": np.ascontiguousarray(cos[kpos].T.astype(np.float16)),
        "sin_k": np.ascontiguousarray((sin[kpos] * sgn).T.astype(np.float16)),
        "maskq": np.ascontiguousarray(maskq.reshape(128, 4 * 512)),
    }
    ci.update(wpack)
    return ci


def _build_runner(nc, n_cores=8):
    """jit the SPMD body once so repeat kernel() calls skip retracing."""
    import jax
    from jax.experimental.shard_map import shard_map
    from jax.sharding import Mesh, NamedSharding, PartitionSpec

    from concourse import bass2jax

    bass2jax.install_neuronx_cc_hook()
    partition_name = (nc.partition_id_tensor.name
                      if nc.partition_id_tensor else None)
    in_names, out_names, out_avals = [], [], []
    for alloc in nc.m.functions[0].allocations:
        if not isinstance(alloc, mybir.MemoryLocationSet):
            continue
        name = alloc.memorylocations[0].name
        if alloc.kind == "ExternalInput":
            if name != partition_name:
                in_names.append(name)
        elif alloc.kind == "ExternalOutput":
            out_names.append(name)
            out_avals.append(jax.core.ShapedArray(
                tuple(alloc.tensor_shape), mybir.dt.np(alloc.dtype)))
    all_in = list(in_names) + list(out_names)
    if partition_name is not None:
        all_in.append(partition_name)

    def _body(*args):
        operands = list(args)
        if partition_name is not None:
            operands.append(bass2jax.partition_id_tensor())
        return tuple(bass2jax._bass_exec_p.bind(
            *operands, out_avals=tuple(out_avals), in_names=tuple(all_in),
            out_names=tuple(out_names), lowering_input_output_aliases=(),
            sim_require_finite=True, sim_require_nnan=True, nc=nc))

    devices = jax.devices()[:n_cores]
    mesh = Mesh(np.asarray(devices), ("core",))
    nspec = (PartitionSpec("core"),)
    sharded = jax.jit(
        shard_map(_body, mesh=mesh,
                  in_specs=nspec * (len(in_names) + len(out_avals)),
                  out_specs=nspec * len(out_avals), check_rep=False),
        keep_unused=True)
    sharding = NamedSharding(mesh, PartitionSpec("core"))
    zeros = [jax.device_put(
        np.zeros((n_cores * a.shape[0], *a.shape[1:]), a.dtype), sharding)
        for a in out_avals]
    return {"fn": sharded, "in_names": in_names, "out_names": out_names,
            "out_avals": out_avals, "sharding": sharding, "zeros": zeros,
            "dev_cache": {}}


def _run_cached(runner, in_maps):
    """Repeat-call path: device-cache replicated tensors by fingerprint."""
    import hashlib

    import jax

    n_cores = len(in_maps)
    args = []
    for name in runner["in_names"]:
        arrs = [np.asarray(in_maps[c][name]) for c in range(n_cores)]
        replicated = all(a is arrs[0] or np.shares_memory(a, arrs[0])
                         for a in arrs)
        if replicated:
            h = hashlib.blake2b(arrs[0].tobytes(), digest_size=16).hexdigest()
            key = (name, h)
            if key not in runner["dev_cache"]:
                runner["dev_cache"] = {k: v for k, v in
                                       runner["dev_cache"].items()
                                       if k[0] != name}
                runner["dev_cache"][key] = jax.device_put(
                    np.concatenate(arrs, axis=0), runner["sharding"])
            args.append(runner["dev_cache"][key])
        else:
            args.append(jax.device_put(np.concatenate(arrs, axis=0),
                                       runner["sharding"]))
    outs = runner["fn"](*args, *runner["zeros"])
    outs = [np.asarray(o) for o in outs]
    return [{name: outs[i].reshape(n_cores, *runner["out_avals"][i].shape)[c]
             for i, name in enumerate(runner["out_names"])}
            for c in range(n_cores)]


def _prep_inputs(x, cos, sin, wq, wk, wv, wo):
    x = np.ascontiguousarray(np.asarray(x, np.float32))
    cos = np.asarray(cos, np.float32)
    sin = np.asarray(sin, np.float32)
    wqT = np.ascontiguousarray(np.asarray(wq, np.float32).T)
    wkT = np.ascontiguousarray(np.asarray(wk, np.float32).T)
    wvT = np.ascontiguousarray(np.asarray(wv, np.float32).T)
    woT = np.ascontiguousarray(np.asarray(wo, np.float32).T)
    xT_full = np.ascontiguousarray(x.transpose(0, 2, 1))
    xh0, xl0 = _hilo(xT_full.reshape(-1, L), SX)
    xTh_full = xh0.reshape(B, DIM, L)
    xTl_full = xl0.reshape(B, DIM, L)
    wpack = _pack_weights(wqT, wkT, wvT, woT)
    return [_core_inputs(xTh_full, xTl_full, cos, sin, wpack, core)
            for core in range(8)]


def kernel(x, cos, sin, wq, wk, wv, wo, _return_results=False):
    if "nc" not in _CACHE:
        _CACHE["nc"] = _build_nc()
    nc = _CACHE["nc"]

    in_maps = _prep_inputs(x, cos, sin, wq, wk, wv, wo)

    res = None
    if not _CACHE.get("ran_once"):
        res = run_bass_kernel_spmd(nc, in_maps, core_ids=list(range(8)))
        results = res.results
        _CACHE["ran_once"] = True
    else:
        if "runner" not in _CACHE:
            try:
                _CACHE["runner"] = _build_runner(nc)
            except Exception:
                _CACHE["runner"] = None
        if _CACHE["runner"] is not None:
            results = _run_cached(_CACHE["runner"], in_maps)
        else:
            res = run_bass_kernel_spmd(nc, in_maps, core_ids=list(range(8)))
            results = res.results

    full = np.zeros((B, L, DIM), np.float32)
    for core in range(8):
        b, chunk = core // 4, core % 4
        full[b, chunk * LQ:(chunk + 1) * LQ] = results[core]["out"]
    if _return_results:
        return full, res
    return full
